# revision 10
# baseline (speedup 1.0000x reference)
"""ButterflyBlock sparse-attention kernel for 8 Trainium2 NeuronCores.

Full inputs in, full output out. The P*B = 32 butterfly blocks are
data-parallel: 4 blocks per core, QKVO weights persistent in SBUF,
chunk gather/scatter done host-side in numpy.

Hardcoded problem shape: x [4, 4096, 1024], D=1024, H=16 heads, dh=64,
CHUNK=256 -> C=16 chunks, pairs a < a^(1<<layer_bit), blocks of L=512.

Schedule: globally software-pipelined emission keeping the PE gap-free.
Attention of block b is interleaved with Q/K/V projections of block b+1
(and deferred Wo groups) as filler work, so the scores->exp->PV chain
never stalls the tensor engine and the PE p-state stays at max clock.
"""

import sys

sys.path.insert(0, "/root/.axon_site/_ro/trn_rl_repo")
sys.path.insert(0, "/opt/trn_rl_repo")

import ml_dtypes
import numpy as np

import concourse.bass as bass
import concourse.bacc as bacc
import concourse.mybir as mybir
import concourse.tile as tile
from concourse.bass_utils import run_bass_kernel_spmd

F32 = mybir.dt.float32
BF16 = mybir.dt.bfloat16

B, N, D = 4, 4096, 1024
H, DH = 16, 64
CHUNK = 256
L = 2 * CHUNK          # 512 tokens per block
NBLK = 4               # blocks per core
NCORES = 8
KC = D // 128          # 8 contraction chunks
LC = L // 128          # 4 token chunks
EXP_FUNC = mybir.ActivationFunctionType.Exp

# v_sb free layout per m-chunk: 16 head-blocks of 128 cols each,
# every head h: [ones(64) | v_h(64)]  (ones-first so the softmax sum S
# lands on PSUM partitions 0:64 where the custom DVE recip can read it)
VW = H * 128           # 2048


def _build_nc(has_bq, has_bk, has_bv):
    nc = bacc.Bacc("TRN2", target_bir_lowering=False, debug=False)

    zt = nc.dram_tensor("zt", [NBLK, D, L], BF16, kind="ExternalInput")
    # wq/wk are dc-major: [128, dc, kc*128] so one DMA chunk unlocks a
    # whole projection output group at cold start
    wq = nc.dram_tensor("wq", [128, KC, D], BF16, kind="ExternalInput")
    wk = nc.dram_tensor("wk", [128, KC, D], BF16, kind="ExternalInput")
    # wv/wo are kc-major (moving operands)
    wv = nc.dram_tensor("wv", [128, KC, D], BF16, kind="ExternalInput")
    wo = nc.dram_tensor("wo", [128, KC, D], BF16, kind="ExternalInput")
    ones = nc.dram_tensor("ones", [128, 64], BF16, kind="ExternalInput")
    y = nc.dram_tensor("y", [NBLK, L, D], BF16, kind="ExternalOutput")
    bq = bk = bv = None
    if has_bq:
        bq = nc.dram_tensor("bq", [128, KC], F32, kind="ExternalInput")
    if has_bk:
        bk = nc.dram_tensor("bk", [128, KC], F32, kind="ExternalInput")
    if has_bv:
        bv = nc.dram_tensor("bv", [128, KC], F32, kind="ExternalInput")

    with tile.TileContext(nc) as tc:
        with (
            tc.tile_pool(name="persist", bufs=1) as pp,
            tc.tile_pool(name="ysb", bufs=3) as ypool,
            tc.tile_pool(name="rsb", bufs=2) as rpool,
            tc.tile_pool(name="scps", bufs=4, space="PSUM") as scps,
            tc.tile_pool(name="mmps", bufs=3, space="PSUM") as mmps,
        ):
            # ---- persistent SBUF tiles -------------------------------
            wq_sb = pp.tile([128, KC, D], BF16, tag="wq")
            wk_sb = pp.tile([128, KC, D], BF16, tag="wk")
            wv_sb = pp.tile([128, KC, D], BF16, tag="wv")
            wo_sb = pp.tile([128, KC, D], BF16, tag="wo")
            zt_sb = [pp.tile([128, KC, L], BF16, tag="zt%d" % i, name="zt%d" % i)
                     for i in range(2)]
            q_sb = [pp.tile([128, KC, L], BF16, tag="q%d" % i, name="q%d" % i)
                    for i in range(2)]
            k_sb = [pp.tile([128, KC, L], BF16, tag="k%d" % i, name="k%d" % i)
                    for i in range(2)]
            v_sb = [pp.tile([128, LC, VW], BF16, tag="v%d" % i, name="v%d" % i)
                    for i in range(2)]
            u_sb = [pp.tile([128, KC, L], BF16, tag="u%d" % i, name="u%d" % i)
                    for i in range(2)]
            p_e = [pp.tile([128, LC, 512], BF16, tag="pe%d" % i, name="pe%d" % i)
                   for i in range(2)]
            p_o = [pp.tile([128, LC, 512], BF16, tag="po%d" % i, name="po%d" % i)
                   for i in range(2)]

            # ---- HAM warmup --------------------------------------------
            # the framework preamble + DMA ring startup keeps the PE idle
            # for ~11us; a dozen throwaway matmuls on a zeroed tile keep it
            # busy through that window so the HAM clock gate is already at
            # 8/8 (2.4 GHz) when the first projection matmul issues
            wu_sb = pp.tile([128, 512], BF16, tag="wu")
            nc.vector.memset(wu_sb[:], 0.0)
            wu_ps = mmps.tile([128, 512], F32, tag="mm")
            for _ in range(12):
                nc.tensor.matmul(
                    wu_ps[:], wu_sb[:, 0:128].opt(), wu_sb[:].opt(),
                    start=True, stop=True,
                )

            bq_sb = bk_sb = bv_sb = None
            if has_bq:
                bq_sb = pp.tile([128, KC], F32, tag="bq")
                nc.sync.dma_start(bq_sb[:], bq[:])
            if has_bk:
                bk_sb = pp.tile([128, KC], F32, tag="bk")
                nc.sync.dma_start(bk_sb[:], bk[:])
            if has_bv:
                bv_sb = pp.tile([128, KC], F32, tag="bv")
                nc.sync.dma_start(bv_sb[:], bv[:])

            # ---- initial DMAs ----------------------------------------
            # zt block 0 per-kc on the gpsimd queue (fine grain so the
            # first projection matmuls start ~1us in); wq dc-chunks on
            # the sync queue.  Remaining weights + zt follow.
            zt_r = [zt[b].rearrange("(kc p) l -> p kc l", p=128)
                    for b in range(NBLK)]
            # zt block 0 per-kc split over the gpsimd + vector queues so the
            # first projection matmuls start as soon as each chunk lands;
            # wq/wk split across the sync + scalar HWDGE rings
            for kc in range(KC):
                eng = nc.gpsimd if kc % 2 == 0 else nc.scalar
                eng.dma_start(zt_sb[0][:, kc, :], zt_r[0][:, kc, :])
            for dc in range(KC):
                eng = nc.sync if dc % 2 == 0 else nc.scalar
                eng.dma_start(wq_sb[:, dc, :], wq[:, dc, :])
            for dc in range(KC):
                eng = nc.sync if dc % 2 == 0 else nc.scalar
                eng.dma_start(wk_sb[:, dc, :], wk[:, dc, :])
            for kc in range(KC):
                nc.sync.dma_start(wv_sb[:, kc, :], wv[:, kc, :])
            for kc in range(KC):
                nc.sync.dma_start(wo_sb[:, kc, :], wo[:, kc, :])
            # ones margins: head-block base (cols h*128+0:64) of every
            # (lc, h); written once per v buffer, never overwritten
            ones_b = bass.AP(
                tensor=ones[:].tensor, offset=ones[:].offset,
                ap=[list(ones[:].ap[0]), [0, H], [1, 64]],
            )
            for i in range(2):
                base = v_sb[i][:]
                for lc in range(LC):
                    dst = bass.AP(
                        tensor=base.tensor, offset=base.offset + lc * VW,
                        ap=[list(base.ap[0]), [128, H], [1, 64]],
                    )
                    nc.sync.dma_start(dst, ones_b)
            # zt block 1 prefetch (buffer 1, no prior reader)
            nc.gpsimd.dma_start(zt_sb[1][:], zt_r[1])

            # ---- emitters --------------------------------------------
            def qk_group(b, dc, which):
                """Q or K projection output-chunk dc of block b."""
                w = wq_sb if which == 0 else wk_sb
                out = q_sb[b % 2] if which == 0 else k_sb[b % 2]
                b_s = bq_sb if which == 0 else bk_sb
                ps = mmps.tile([128, L], F32, tag="mm")
                for kc in range(KC):
                    nc.tensor.matmul(
                        ps[:],
                        w[:, dc, kc * 128:(kc + 1) * 128].opt(),
                        zt_sb[b % 2][:, kc, :].opt(),
                        start=(kc == 0),
                        stop=(kc == KC - 1),
                    )
                if b_s is not None:
                    nc.scalar.activation(
                        out[:, dc, :], ps[:],
                        mybir.ActivationFunctionType.Identity,
                        bias=b_s[:, dc:dc + 1], scale=1.0,
                    )
                else:
                    nc.vector.tensor_copy(out[:, dc, :], ps[:])

            def v_group(b, g):
                """V projection group g=(lc, nh) of block b."""
                lc, nh = g // 2, g % 2
                ps = mmps.tile([128, 512], F32, tag="mm")
                for kc in range(KC):
                    nc.tensor.matmul(
                        ps[:],
                        zt_sb[b % 2][:, kc, lc * 128:(lc + 1) * 128].opt(),
                        wv_sb[:, kc, nh * 512:(nh + 1) * 512].opt(),
                        start=(kc == 0),
                        stop=(kc == KC - 1),
                    )
                # heads nh*8..nh*8+7, 64 v cols each at block offset +64
                base = v_sb[b % 2][:]
                dst = bass.AP(
                    tensor=base.tensor,
                    offset=base.offset + lc * VW + nh * 1024 + 64,
                    ap=[list(base.ap[0]), [128, 8], [1, 64]],
                )
                src = bass.AP(
                    tensor=ps.tensor, offset=ps[:].offset,
                    ap=[list(ps[:].ap[0]), [64, 8], [1, 64]],
                )
                nc.vector.tensor_copy(dst, src)

            y_rr = [0]

            def wo_group(b, g, split_y=False, y_on_scalar=False):
                """Output projection group g=(lc, eh) of block b.
                dc ascends so the accumulation chases the last u chunks.
                y leaves as bf16, round-robined over two HWDGE rings so the
                final block's writes drain ~2x faster."""
                lc, eh = g // 2, g % 2
                ps = mmps.tile([128, 512], F32, tag="mm")
                for dc in range(KC):
                    nc.tensor.matmul(
                        ps[:],
                        u_sb[b % 2][:, dc, lc * 128:(lc + 1) * 128].opt(),
                        wo_sb[:, dc, eh * 512:(eh + 1) * 512].opt(),
                        start=(dc == 0),
                        stop=(dc == KC - 1),
                    )
                y_sb = ypool.tile([128, 512], BF16, tag="y")
                halves = (0, 256, 512) if split_y else (0, 512)
                for lo, hi in zip(halves, halves[1:]):
                    if y_on_scalar:
                        nc.scalar.copy(y_sb[:, lo:hi], ps[:, lo:hi])
                    else:
                        nc.vector.tensor_copy(y_sb[:, lo:hi], ps[:, lo:hi])
                    eng = (nc.sync, nc.gpsimd)[y_rr[0] % 2]
                    y_rr[0] += 1
                    eng.dma_start(
                        y[b, lc * 128:(lc + 1) * 128,
                          eh * 512 + lo:eh * 512 + hi],
                        y_sb[:, lo:hi],
                    )

            def sc_mg(b, c, mg):
                """Scores chunk-group mg (key chunks 2mg, 2mg+1) for head
                pair c of block b.  Each (chunk, parity) gets its own
                single-bank PSUM tile so the exp consumer releases banks at
                512-col granularity instead of 1024 -- halves the PE stall
                on score-tile reuse."""
                out = []
                for i in range(2):
                    mc = 2 * mg + i
                    t_e = scps.tile([128, 512], F32, tag="sc")
                    t_o = scps.tile([128, 512], F32, tag="sc")
                    for par, t in ((0, t_e), (1, t_o)):
                        half = par * 64
                        nc.tensor.matmul(
                            t[:],
                            k_sb[b % 2][half:half + 64, c,
                                        mc * 128:(mc + 1) * 128].opt(),
                            q_sb[b % 2][half:half + 64, c, :].opt(),
                            start=True, stop=True,
                        )
                    out.append((t_e, t_o))
                return out

            def pv(b, c, par):
                """PV matmul for head h=2c+par; returns the PSUM tile
                holding S on rows 0:64 and u on rows 64:128."""
                h = 2 * c + par
                p_t = (p_e if par == 0 else p_o)[c % 2]
                ps = mmps.tile([128, 512], F32, tag="mm")
                for mc in range(LC):
                    nc.tensor.matmul(
                        ps[:],
                        v_sb[b % 2][:, mc, h * 128:(h + 1) * 128].opt(),
                        p_t[:, mc, :].opt(),
                        start=(mc == 0), stop=(mc == LC - 1),
                    )
                return ps

            def att_phase(b, fillers):
                fi = iter(fillers)

                def F():
                    f = next(fi, None)
                    if f is not None:
                        f()

                ub = u_sb[b % 2]

                def norm(c, par, ps):
                    r = rpool.tile([64, 512], F32, tag="r%d" % par)
                    nc.vector.reciprocal_approx_fast(r[0:64, :], ps[0:64, :])
                    if par == 0:
                        tmp = rpool.tile([64, 512], F32, tag="tmp")
                        nc.vector.tensor_copy(tmp[0:64, :], ps[64:128, :])
                        nc.vector.tensor_mul(
                            ub[0:64, c, :], tmp[0:64, :], r[0:64, :])
                    else:
                        nc.vector.tensor_mul(
                            ub[64:128, c, :], ps[64:128, :], r[0:64, :])
                        if has_bv:
                            nc.vector.tensor_scalar_add(
                                ub[:, c, :], ub[:, c, :], bv_sb[:, c:c + 1])

                for c in range(H // 2):
                    pe, po = (p_e[c % 2], p_o[c % 2])
                    g0 = sc_mg(b, c, 0)
                    for i in range(2):
                        nc.scalar.activation(pe[:, i, :], g0[i][0][:],
                                             EXP_FUNC)
                    for i in range(2):
                        nc.scalar.activation(po[:, i, :], g0[i][1][:],
                                             EXP_FUNC)
                    F()
                    g1 = sc_mg(b, c, 1)
                    for i in range(2):
                        nc.scalar.activation(pe[:, 2 + i, :], g1[i][0][:],
                                             EXP_FUNC)
                    for i in range(2):
                        nc.scalar.activation(po[:, 2 + i, :], g1[i][1][:],
                                             EXP_FUNC)
                    if c > 0:
                        norm(c - 1, 0, pv(b, c - 1, 0))
                    F()
                    if c > 0:
                        norm(c - 1, 1, pv(b, c - 1, 1))
                # epilogue: last head pair
                norm(7, 0, pv(b, 7, 0))
                F()
                norm(7, 1, pv(b, 7, 1))
                for f in fi:   # drain any leftover fillers
                    f()

            # ---- global emission order -------------------------------
            # cold: block-0 projections (DMA-paced)
            for dc in range(KC):
                qk_group(0, dc, 0)
            for dc in range(KC):
                qk_group(0, dc, 1)
            for g in range(8):
                v_group(0, g)
            # zt0's last reader (V0) is emitted; buffer 0 may now be
            # refilled with block 2 (emission order IS the dep order)
            nc.gpsimd.dma_start(zt_sb[0][:], zt_r[2])

            # att0 || [Q1, K1]
            att_phase(0, [lambda dc=dc: qk_group(1, dc, 0) for dc in range(KC)]
                      + [lambda dc=dc: qk_group(1, dc, 1) for dc in range(KC)])
            for g in range(8):
                v_group(1, g)
            # zt1's last reader (V1) emitted; refill buffer 1 with block 3
            nc.gpsimd.dma_start(zt_sb[1][:], zt_r[3])
            for g in range(8):
                wo_group(0, g, y_on_scalar=True)

            # att1 || [Q2, K2]
            att_phase(1, [lambda dc=dc: qk_group(2, dc, 0) for dc in range(KC)]
                      + [lambda dc=dc: qk_group(2, dc, 1) for dc in range(KC)])
            for g in range(8):
                v_group(2, g)
            for g in range(5):
                wo_group(1, g, y_on_scalar=True)

            # att2 || [Q3, V3, K3 g0/g1] -- the trailing K3 groups land in
            # the epilogue/drain slots, just ahead of att3's first scores
            att_phase(2, [lambda dc=dc: qk_group(3, dc, 0) for dc in range(KC)]
                      + [lambda g=g: v_group(3, g) for g in range(8)]
                      + [lambda dc=dc: qk_group(3, dc, 1) for dc in range(2)])

            # att3 || [Wo1 spill, K3 rest, Wo2] -- 17 fillers so one lands
            # in the epilogue slot, pushing Wo3's first u-chunk-7 ldweights
            # past the final normalize muls
            # at most 3 Wo1 spill groups: they must all be consumed before
            # att3's first u-normalize write (sets 1 share the u buffer)
            att_phase(3, [lambda g=g: wo_group(1, g) for g in range(5, 8)]
                      + [lambda dc=dc: qk_group(3, dc, 1)
                         for dc in range(2, KC)]
                      + [lambda g=g: wo_group(2, g, y_on_scalar=(g >= 4))
                         for g in range(8)])
            for g in range(8):
                wo_group(3, g, split_y=True, y_on_scalar=True)

    nc.finalize()
    return nc


_NC_CACHE = {}


def _get_nc(flags):
    if flags not in _NC_CACHE:
        _NC_CACHE[flags] = _build_nc(*flags)
    return _NC_CACHE[flags]


def _prep(x, Wq, bq, Wk, bk, Wv, bv, Wo, bo, layer_bit):
    x = np.asarray(x, dtype=np.float32)
    C = N // CHUNK
    ids = np.arange(C)
    partner = ids ^ (1 << int(layer_bit))
    a_idx = ids[ids < partner]
    b_idx = partner[ids < partner]
    P = a_idx.shape[0]

    xr = x.reshape(B, C, CHUNK, D)
    blocks = np.concatenate([xr[:, a_idx], xr[:, b_idx]], axis=2)  # [B,P,L,D]
    blocks = np.ascontiguousarray(
        blocks.transpose(1, 0, 3, 2).reshape(P * B, D, L).astype(ml_dtypes.bfloat16)
    )  # z^T per block
    scale = np.float32(1.0 / np.sqrt(DH))

    def chunkify(vec):  # [D] -> [128, KC] chunk-major per-partition scalars
        return np.ascontiguousarray(
            np.asarray(vec, np.float32).reshape(KC, 128).T
        )

    bf = ml_dtypes.bfloat16

    def dc_major(w):  # [D, D] -> [128, dc, kc*128]
        a = np.asarray(w, np.float32).reshape(KC, 128, KC, 128)
        return np.ascontiguousarray(
            a.transpose(1, 2, 0, 3).reshape(128, KC, D).astype(bf))

    def kc_major(w):  # [D, D] -> [128, kc, D]
        a = np.asarray(w, np.float32).reshape(KC, 128, D)
        return np.ascontiguousarray(a.transpose(1, 0, 2).astype(bf))

    base = {
        "wq": dc_major(np.asarray(Wq, np.float32) * scale),
        "wk": dc_major(Wk),
        "wv": kc_major(Wv),
        "wo": kc_major(Wo),
        "ones": np.ones((128, 64), bf),
    }
    has_bq = bool(np.any(np.asarray(bq))) if bq is not None else False
    has_bk = bool(np.any(np.asarray(bk))) if bk is not None else False
    has_bv = bool(np.any(np.asarray(bv))) if bv is not None else False
    if has_bq:
        base["bq"] = chunkify(np.asarray(bq, np.float32) * scale)
    if has_bk:
        base["bk"] = chunkify(bk)
    if has_bv:
        base["bv"] = chunkify(bv)

    in_maps = []
    for core in range(NCORES):
        m = dict(base)
        m["zt"] = blocks[core * NBLK:(core + 1) * NBLK]
        in_maps.append(m)
    return in_maps, (has_bq, has_bk, has_bv), (a_idx, b_idx, P)


def _gather(results, idxs, bo):
    a_idx, b_idx, P = idxs
    yb = np.concatenate([np.asarray(r["y"], np.float32) for r in results],
                        axis=0)  # [P*B, L, D]
    yb = yb.reshape(P, B, 2, CHUNK, D)
    out = np.empty((B, N // CHUNK, CHUNK, D), np.float32)
    out[:, a_idx] = yb[:, :, 0].transpose(1, 0, 2, 3)
    out[:, b_idx] = yb[:, :, 1].transpose(1, 0, 2, 3)
    out = out.reshape(B, N, D)
    bo = np.asarray(bo, np.float32) if bo is not None else None
    if bo is not None and np.any(bo):
        out = out + bo
    return out


def _run(inputs, trace=False):
    in_maps, flags, idxs = _prep(
        inputs["x"], inputs["Wq"], inputs.get("bq"), inputs["Wk"],
        inputs.get("bk"), inputs["Wv"], inputs.get("bv"), inputs["Wo"],
        inputs.get("bo"), inputs["layer_bit"],
    )
    nc = _get_nc(flags)
    res = run_bass_kernel_spmd(nc, in_maps, list(range(NCORES)), trace=trace)
    out = _gather(res.results, idxs, inputs.get("bo"))
    return out, res


def kernel(**inputs):
    out, _ = _run(inputs, trace=False)
    return out


def kernel_traced(**inputs):
    out, res = _run(inputs, trace=True)
    return out, res



# revision 17
# speedup vs baseline: 1.0244x; 1.0244x over previous
"""ButterflyBlock sparse-attention kernel for 8 Trainium2 NeuronCores.

Full inputs in, full output out. The P*B = 32 butterfly blocks are
data-parallel: 4 blocks per core, QKVO weights persistent in SBUF,
chunk gather/scatter done host-side in numpy.

Hardcoded problem shape: x [4, 4096, 1024], D=1024, H=16 heads, dh=64,
CHUNK=256 -> C=16 chunks, pairs a < a^(1<<layer_bit), blocks of L=512.

Schedule: globally software-pipelined emission keeping the PE gap-free.
Attention of block b is interleaved with Q/K/V projections of block b+1
(and deferred Wo groups) as filler work, so the scores->exp->PV chain
never stalls the tensor engine and the PE p-state stays at max clock.
"""

import sys

sys.path.insert(0, "/root/.axon_site/_ro/trn_rl_repo")
sys.path.insert(0, "/opt/trn_rl_repo")

import ml_dtypes
import numpy as np

import concourse.bass as bass
import concourse.bacc as bacc
import concourse.mybir as mybir
import concourse.tile as tile
from concourse.bass_utils import run_bass_kernel_spmd

F32 = mybir.dt.float32
BF16 = mybir.dt.bfloat16

B, N, D = 4, 4096, 1024
H, DH = 16, 64
CHUNK = 256
L = 2 * CHUNK          # 512 tokens per block
NBLK = 4               # blocks per core
NCORES = 8
KC = D // 128          # 8 contraction chunks
LC = L // 128          # 4 token chunks
EXP_FUNC = mybir.ActivationFunctionType.Exp

# v_sb free layout per m-chunk: 16 head-blocks of 128 cols each,
# every head h: [ones(64) | v_h(64)]  (ones-first so the softmax sum S
# lands on PSUM partitions 0:64 where the custom DVE recip can read it)
VW = H * 128           # 2048


def _build_nc(has_bq, has_bk, has_bv):
    nc = bacc.Bacc("TRN2", target_bir_lowering=False, debug=False)

    zt = nc.dram_tensor("zt", [NBLK, D, L], BF16, kind="ExternalInput")
    # wq/wk are dc-major: [128, dc, kc*128] so one DMA chunk unlocks a
    # whole projection output group at cold start
    wq = nc.dram_tensor("wq", [128, KC, D], BF16, kind="ExternalInput")
    wk = nc.dram_tensor("wk", [128, KC, D], BF16, kind="ExternalInput")
    # wv/wo are kc-major (moving operands)
    wv = nc.dram_tensor("wv", [128, KC, D], BF16, kind="ExternalInput")
    wo = nc.dram_tensor("wo", [128, KC, D], BF16, kind="ExternalInput")
    ones = nc.dram_tensor("ones", [128, 64], BF16, kind="ExternalInput")
    y = nc.dram_tensor("y", [NBLK, L, D], BF16, kind="ExternalOutput")
    bq = bk = bv = None
    if has_bq:
        bq = nc.dram_tensor("bq", [128, KC], F32, kind="ExternalInput")
    if has_bk:
        bk = nc.dram_tensor("bk", [128, KC], F32, kind="ExternalInput")
    if has_bv:
        bv = nc.dram_tensor("bv", [128, KC], F32, kind="ExternalInput")

    with tile.TileContext(nc) as tc:
        with (
            tc.tile_pool(name="persist", bufs=1) as pp,
            tc.tile_pool(name="ysb", bufs=3) as ypool,
            tc.tile_pool(name="rsb", bufs=2) as rpool,
            tc.tile_pool(name="scps", bufs=3, space="PSUM") as scps,
            tc.tile_pool(name="mmps", bufs=2, space="PSUM") as mmps,
        ):
            # ---- persistent SBUF tiles -------------------------------
            wq_sb = pp.tile([128, KC, D], BF16, tag="wq")
            wk_sb = pp.tile([128, KC, D], BF16, tag="wk")
            wv_sb = pp.tile([128, KC, D], BF16, tag="wv")
            wo_sb = pp.tile([128, KC, D], BF16, tag="wo")
            zt_sb = [pp.tile([128, KC, L], BF16, tag="zt%d" % i, name="zt%d" % i)
                     for i in range(2)]
            q_sb = [pp.tile([128, KC, L], BF16, tag="q%d" % i, name="q%d" % i)
                    for i in range(2)]
            k_sb = [pp.tile([128, KC, L], BF16, tag="k%d" % i, name="k%d" % i)
                    for i in range(2)]
            v_sb = [pp.tile([128, LC, VW], BF16, tag="v%d" % i, name="v%d" % i)
                    for i in range(2)]
            u_sb = [pp.tile([128, KC, L], BF16, tag="u%d" % i, name="u%d" % i)
                    for i in range(2)]
            p_e = [pp.tile([128, LC, 512], BF16, tag="pe%d" % i, name="pe%d" % i)
                   for i in range(2)]
            p_o = [pp.tile([128, LC, 512], BF16, tag="po%d" % i, name="po%d" % i)
                   for i in range(2)]

            # ---- HAM warmup --------------------------------------------
            # the framework preamble + DMA ring startup keeps the PE idle
            # for ~11us; throwaway matmuls on a zeroed tile keep it busy
            # through that window so the HAM clock gate is already at 8/8
            # (2.4 GHz) when the first projection matmul issues.  More are
            # interleaved into the DMA-paced first projection below so the
            # PE never idles long enough to re-throttle.
            wu_sb = pp.tile([128, 512], BF16, tag="wu")
            nc.vector.memset(wu_sb[:], 0.0)

            def warm(n):
                wu_ps = mmps.tile([128, 512], F32, tag="mm")
                for _ in range(n):
                    nc.tensor.matmul(
                        wu_ps[:], wu_sb[:, 0:128].opt(), wu_sb[:].opt(),
                        start=True, stop=True,
                    )

            warm(10)

            bq_sb = bk_sb = bv_sb = None
            if has_bq:
                bq_sb = pp.tile([128, KC], F32, tag="bq")
                nc.sync.dma_start(bq_sb[:], bq[:])
            if has_bk:
                bk_sb = pp.tile([128, KC], F32, tag="bk")
                nc.sync.dma_start(bk_sb[:], bk[:])
            if has_bv:
                bv_sb = pp.tile([128, KC], F32, tag="bv")
                nc.sync.dma_start(bv_sb[:], bv[:])

            # ---- initial DMAs ----------------------------------------
            # zt block 0 per-kc on the gpsimd queue (fine grain so the
            # first projection matmuls start ~1us in); wq dc-chunks on
            # the sync queue.  Remaining weights + zt follow.
            zt_r = [zt[b].rearrange("(kc p) l -> p kc l", p=128)
                    for b in range(NBLK)]
            # zt block 0 split over the gpsimd + sync rings (evens/odds) so
            # the whole block lands in ~half the time; wq follows on sync
            # (paces the Q output groups), wk rides scalar in parallel
            for kc in range(0, KC, 2):
                nc.gpsimd.dma_start(zt_sb[0][:, kc, :], zt_r[0][:, kc, :])
            for kc in range(1, KC, 2):
                nc.sync.dma_start(zt_sb[0][:, kc, :], zt_r[0][:, kc, :])
            for dc in range(KC):
                nc.sync.dma_start(wq_sb[:, dc, :], wq[:, dc, :])
            for dc in range(KC):
                nc.scalar.dma_start(wk_sb[:, dc, :], wk[:, dc, :])
            for kc in range(KC):
                nc.sync.dma_start(wv_sb[:, kc, :], wv[:, kc, :])
            for kc in range(KC):
                nc.sync.dma_start(wo_sb[:, kc, :], wo[:, kc, :])
            # ones margins: head-block base (cols h*128+0:64) of every
            # (lc, h); written once per v buffer, never overwritten
            ones_b = bass.AP(
                tensor=ones[:].tensor, offset=ones[:].offset,
                ap=[list(ones[:].ap[0]), [0, H], [1, 64]],
            )
            for i in range(2):
                base = v_sb[i][:]
                for lc in range(LC):
                    dst = bass.AP(
                        tensor=base.tensor, offset=base.offset + lc * VW,
                        ap=[list(base.ap[0]), [128, H], [1, 64]],
                    )
                    nc.sync.dma_start(dst, ones_b)
            # zt block 1 prefetch (buffer 1, no prior reader)
            nc.gpsimd.dma_start(zt_sb[1][:], zt_r[1])

            # ---- emitters --------------------------------------------
            def qk_group(b, dc, which):
                """Q or K projection output-chunk dc of block b."""
                w = wq_sb if which == 0 else wk_sb
                out = q_sb[b % 2] if which == 0 else k_sb[b % 2]
                b_s = bq_sb if which == 0 else bk_sb
                ps = mmps.tile([128, L], F32, tag="mm")
                for kc in range(KC):
                    nc.tensor.matmul(
                        ps[:],
                        w[:, dc, kc * 128:(kc + 1) * 128].opt(),
                        zt_sb[b % 2][:, kc, :].opt(),
                        start=(kc == 0),
                        stop=(kc == KC - 1),
                    )
                if b_s is not None:
                    nc.scalar.activation(
                        out[:, dc, :], ps[:],
                        mybir.ActivationFunctionType.Identity,
                        bias=b_s[:, dc:dc + 1], scale=1.0,
                    )
                else:
                    nc.vector.tensor_copy(out[:, dc, :], ps[:])

            def v_group(b, g):
                """V projection group g=(lc, nh) of block b."""
                lc, nh = g // 2, g % 2
                ps = mmps.tile([128, 512], F32, tag="mm")
                for kc in range(KC):
                    nc.tensor.matmul(
                        ps[:],
                        zt_sb[b % 2][:, kc, lc * 128:(lc + 1) * 128].opt(),
                        wv_sb[:, kc, nh * 512:(nh + 1) * 512].opt(),
                        start=(kc == 0),
                        stop=(kc == KC - 1),
                    )
                # heads nh*8..nh*8+7, 64 v cols each at block offset +64
                base = v_sb[b % 2][:]
                dst = bass.AP(
                    tensor=base.tensor,
                    offset=base.offset + lc * VW + nh * 1024 + 64,
                    ap=[list(base.ap[0]), [128, 8], [1, 64]],
                )
                src = bass.AP(
                    tensor=ps.tensor, offset=ps[:].offset,
                    ap=[list(ps[:].ap[0]), [64, 8], [1, 64]],
                )
                nc.vector.tensor_copy(dst, src)

            y_rr = [0]

            def wo_group(b, g, split_y=False, y_on_scalar=False):
                """Output projection group g=(lc, eh) of block b.
                dc ascends so the accumulation chases the last u chunks.
                y leaves as bf16, round-robined over two HWDGE rings so the
                final block's writes drain ~2x faster."""
                lc, eh = g // 2, g % 2
                ps = mmps.tile([128, 512], F32, tag="mm")
                for dc in range(KC):
                    nc.tensor.matmul(
                        ps[:],
                        u_sb[b % 2][:, dc, lc * 128:(lc + 1) * 128].opt(),
                        wo_sb[:, dc, eh * 512:(eh + 1) * 512].opt(),
                        start=(dc == 0),
                        stop=(dc == KC - 1),
                    )
                y_sb = ypool.tile([128, 512], BF16, tag="y")
                halves = (0, 256, 512) if split_y else (0, 512)
                for lo, hi in zip(halves, halves[1:]):
                    if y_on_scalar:
                        nc.scalar.copy(y_sb[:, lo:hi], ps[:, lo:hi])
                    else:
                        nc.vector.tensor_copy(y_sb[:, lo:hi], ps[:, lo:hi])
                    eng = (nc.sync, nc.gpsimd)[y_rr[0] % 2]
                    y_rr[0] += 1
                    eng.dma_start(
                        y[b, lc * 128:(lc + 1) * 128,
                          eh * 512 + lo:eh * 512 + hi],
                        y_sb[:, lo:hi],
                    )

            def sc_mg(b, c, mg):
                """Scores chunk-group mg (key chunks 2mg, 2mg+1) for head
                pair c of block b; emits the even-parity exp eagerly and
                the rest after mg1 so the scalar queue drains e-major."""
                t_e = scps.tile([128, 2, 512], F32, tag="sc")
                t_o = scps.tile([128, 2, 512], F32, tag="sc")
                for i in range(2):
                    mc = 2 * mg + i
                    for par, t in ((0, t_e), (1, t_o)):
                        half = par * 64
                        nc.tensor.matmul(
                            t[:, i, :],
                            k_sb[b % 2][half:half + 64, c,
                                        mc * 128:(mc + 1) * 128].opt(),
                            q_sb[b % 2][half:half + 64, c, :].opt(),
                            start=True, stop=True,
                        )
                return t_e, t_o

            def pv(b, c, par):
                """PV matmul for head h=2c+par; returns the PSUM tile
                holding S on rows 0:64 and u on rows 64:128."""
                h = 2 * c + par
                p_t = (p_e if par == 0 else p_o)[c % 2]
                ps = mmps.tile([128, 512], F32, tag="mm")
                for mc in range(LC):
                    nc.tensor.matmul(
                        ps[:],
                        v_sb[b % 2][:, mc, h * 128:(h + 1) * 128].opt(),
                        p_t[:, mc, :].opt(),
                        start=(mc == 0), stop=(mc == LC - 1),
                    )
                return ps

            def att_phase(b, fillers):
                fi = iter(fillers)

                def F():
                    f = next(fi, None)
                    if f is not None:
                        f()

                ub = u_sb[b % 2]

                def norm(c, par, ps):
                    r = rpool.tile([64, 512], F32, tag="r%d" % par)
                    nc.vector.reciprocal_approx_fast(r[0:64, :], ps[0:64, :])
                    if par == 0:
                        tmp = rpool.tile([64, 512], F32, tag="tmp")
                        nc.vector.tensor_copy(tmp[0:64, :], ps[64:128, :])
                        nc.vector.tensor_mul(
                            ub[0:64, c, :], tmp[0:64, :], r[0:64, :])
                    else:
                        nc.vector.tensor_mul(
                            ub[64:128, c, :], ps[64:128, :], r[0:64, :])
                        if has_bv:
                            nc.vector.tensor_scalar_add(
                                ub[:, c, :], ub[:, c, :], bv_sb[:, c:c + 1])

                for c in range(H // 2):
                    pe, po = (p_e[c % 2], p_o[c % 2])
                    t_e0, t_o0 = sc_mg(b, c, 0)
                    nc.scalar.activation(pe[:, 0:2, :], t_e0[:], EXP_FUNC)
                    F()
                    t_e1, t_o1 = sc_mg(b, c, 1)
                    nc.scalar.activation(pe[:, 2:4, :], t_e1[:], EXP_FUNC)
                    nc.scalar.activation(po[:, 0:2, :], t_o0[:], EXP_FUNC)
                    nc.scalar.activation(po[:, 2:4, :], t_o1[:], EXP_FUNC)
                    if c > 0:
                        norm(c - 1, 0, pv(b, c - 1, 0))
                    F()
                    if c > 0:
                        norm(c - 1, 1, pv(b, c - 1, 1))
                # epilogue: last head pair
                norm(7, 0, pv(b, 7, 0))
                F()
                norm(7, 1, pv(b, 7, 1))
                for f in fi:   # drain any leftover fillers
                    f()

            # ---- global emission order -------------------------------
            # cold: block-0 projections (DMA-paced); a few extra warmup
            # matmuls fill the early DMA-arrival gaps so the HAM never
            # sees an idle window
            for dc in range(KC):
                qk_group(0, dc, 0)
                if dc == 0:
                    warm(4)
                elif dc == 1:
                    warm(2)
            for dc in range(KC):
                qk_group(0, dc, 1)
            for g in range(8):
                v_group(0, g)
            # zt0's last reader (V0) is emitted; buffer 0 may now be
            # refilled with block 2 (emission order IS the dep order)
            nc.gpsimd.dma_start(zt_sb[0][:], zt_r[2])

            # att0 || [Q1, K1]
            att_phase(0, [lambda dc=dc: qk_group(1, dc, 0) for dc in range(KC)]
                      + [lambda dc=dc: qk_group(1, dc, 1) for dc in range(KC)])
            for g in range(8):
                v_group(1, g)
            # zt1's last reader (V1) emitted; refill buffer 1 with block 3
            nc.gpsimd.dma_start(zt_sb[1][:], zt_r[3])
            for g in range(8):
                wo_group(0, g, y_on_scalar=True)

            # att1 || [Q2, K2]
            att_phase(1, [lambda dc=dc: qk_group(2, dc, 0) for dc in range(KC)]
                      + [lambda dc=dc: qk_group(2, dc, 1) for dc in range(KC)])
            for g in range(8):
                v_group(2, g)
            for g in range(5):
                wo_group(1, g, y_on_scalar=True)

            # att2 || [Q3, V3, K3 g0/g1] -- the trailing K3 groups land in
            # the epilogue/drain slots, just ahead of att3's first scores
            att_phase(2, [lambda dc=dc: qk_group(3, dc, 0) for dc in range(KC)]
                      + [lambda g=g: v_group(3, g) for g in range(8)]
                      + [lambda dc=dc: qk_group(3, dc, 1) for dc in range(2)])

            # att3 || [Wo1 spill, K3 rest, Wo2] -- 17 fillers so one lands
            # in the epilogue slot, pushing Wo3's first u-chunk-7 ldweights
            # past the final normalize muls
            # at most 3 Wo1 spill groups: they must all be consumed before
            # att3's first u-normalize write (sets 1 share the u buffer)
            att_phase(3, [lambda g=g: wo_group(1, g) for g in range(5, 8)]
                      + [lambda dc=dc: qk_group(3, dc, 1)
                         for dc in range(2, KC)]
                      + [lambda g=g: wo_group(2, g, y_on_scalar=(g >= 4))
                         for g in range(8)])
            for g in range(8):
                wo_group(3, g, split_y=True, y_on_scalar=True)

    nc.finalize()
    return nc


_NC_CACHE = {}


def _get_nc(flags):
    if flags not in _NC_CACHE:
        _NC_CACHE[flags] = _build_nc(*flags)
    return _NC_CACHE[flags]


def _prep(x, Wq, bq, Wk, bk, Wv, bv, Wo, bo, layer_bit):
    x = np.asarray(x, dtype=np.float32)
    C = N // CHUNK
    ids = np.arange(C)
    partner = ids ^ (1 << int(layer_bit))
    a_idx = ids[ids < partner]
    b_idx = partner[ids < partner]
    P = a_idx.shape[0]

    xr = x.reshape(B, C, CHUNK, D)
    blocks = np.concatenate([xr[:, a_idx], xr[:, b_idx]], axis=2)  # [B,P,L,D]
    blocks = np.ascontiguousarray(
        blocks.transpose(1, 0, 3, 2).reshape(P * B, D, L).astype(ml_dtypes.bfloat16)
    )  # z^T per block
    scale = np.float32(1.0 / np.sqrt(DH))

    def chunkify(vec):  # [D] -> [128, KC] chunk-major per-partition scalars
        return np.ascontiguousarray(
            np.asarray(vec, np.float32).reshape(KC, 128).T
        )

    bf = ml_dtypes.bfloat16

    def dc_major(w):  # [D, D] -> [128, dc, kc*128]
        a = np.asarray(w, np.float32).reshape(KC, 128, KC, 128)
        return np.ascontiguousarray(
            a.transpose(1, 2, 0, 3).reshape(128, KC, D).astype(bf))

    def kc_major(w):  # [D, D] -> [128, kc, D]
        a = np.asarray(w, np.float32).reshape(KC, 128, D)
        return np.ascontiguousarray(a.transpose(1, 0, 2).astype(bf))

    base = {
        "wq": dc_major(np.asarray(Wq, np.float32) * scale),
        "wk": dc_major(Wk),
        "wv": kc_major(Wv),
        "wo": kc_major(Wo),
        "ones": np.ones((128, 64), bf),
    }
    has_bq = bool(np.any(np.asarray(bq))) if bq is not None else False
    has_bk = bool(np.any(np.asarray(bk))) if bk is not None else False
    has_bv = bool(np.any(np.asarray(bv))) if bv is not None else False
    if has_bq:
        base["bq"] = chunkify(np.asarray(bq, np.float32) * scale)
    if has_bk:
        base["bk"] = chunkify(bk)
    if has_bv:
        base["bv"] = chunkify(bv)

    in_maps = []
    for core in range(NCORES):
        m = dict(base)
        m["zt"] = blocks[core * NBLK:(core + 1) * NBLK]
        in_maps.append(m)
    return in_maps, (has_bq, has_bk, has_bv), (a_idx, b_idx, P)


def _gather(results, idxs, bo):
    a_idx, b_idx, P = idxs
    yb = np.concatenate([np.asarray(r["y"], np.float32) for r in results],
                        axis=0)  # [P*B, L, D]
    yb = yb.reshape(P, B, 2, CHUNK, D)
    out = np.empty((B, N // CHUNK, CHUNK, D), np.float32)
    out[:, a_idx] = yb[:, :, 0].transpose(1, 0, 2, 3)
    out[:, b_idx] = yb[:, :, 1].transpose(1, 0, 2, 3)
    out = out.reshape(B, N, D)
    bo = np.asarray(bo, np.float32) if bo is not None else None
    if bo is not None and np.any(bo):
        out = out + bo
    return out


def _run(inputs, trace=False):
    in_maps, flags, idxs = _prep(
        inputs["x"], inputs["Wq"], inputs.get("bq"), inputs["Wk"],
        inputs.get("bk"), inputs["Wv"], inputs.get("bv"), inputs["Wo"],
        inputs.get("bo"), inputs["layer_bit"],
    )
    nc = _get_nc(flags)
    res = run_bass_kernel_spmd(nc, in_maps, list(range(NCORES)), trace=trace)
    out = _gather(res.results, idxs, inputs.get("bo"))
    return out, res


def kernel(**inputs):
    out, _ = _run(inputs, trace=False)
    return out


def kernel_traced(**inputs):
    out, res = _run(inputs, trace=True)
    return out, res



# revision 20
# speedup vs baseline: 1.0328x; 1.0082x over previous
"""ButterflyBlock sparse-attention kernel for 8 Trainium2 NeuronCores.

Full inputs in, full output out. The P*B = 32 butterfly blocks are
data-parallel: 4 blocks per core, QKVO weights persistent in SBUF,
chunk gather/scatter done host-side in numpy.

Hardcoded problem shape: x [4, 4096, 1024], D=1024, H=16 heads, dh=64,
CHUNK=256 -> C=16 chunks, pairs a < a^(1<<layer_bit), blocks of L=512.

Schedule: globally software-pipelined emission keeping the PE gap-free.
Attention of block b is interleaved with Q/K/V projections of block b+1
(and deferred Wo groups) as filler work, so the scores->exp->PV chain
never stalls the tensor engine and the PE p-state stays at max clock.
"""

import sys

sys.path.insert(0, "/root/.axon_site/_ro/trn_rl_repo")
sys.path.insert(0, "/opt/trn_rl_repo")

import ml_dtypes
import numpy as np

import concourse.bass as bass
import concourse.bacc as bacc
import concourse.mybir as mybir
import concourse.tile as tile
from concourse.bass_utils import run_bass_kernel_spmd

F32 = mybir.dt.float32
BF16 = mybir.dt.bfloat16

B, N, D = 4, 4096, 1024
H, DH = 16, 64
CHUNK = 256
L = 2 * CHUNK          # 512 tokens per block
NBLK = 4               # blocks per core
NCORES = 8
KC = D // 128          # 8 contraction chunks
LC = L // 128          # 4 token chunks
EXP_FUNC = mybir.ActivationFunctionType.Exp

# v_sb free layout per m-chunk: 16 head-blocks of 128 cols each,
# every head h: [ones(64) | v_h(64)]  (ones-first so the softmax sum S
# lands on PSUM partitions 0:64 where the custom DVE recip can read it)
VW = H * 128           # 2048


def _build_nc(has_bq, has_bk, has_bv):
    nc = bacc.Bacc("TRN2", target_bir_lowering=False, debug=False)

    zt = nc.dram_tensor("zt", [NBLK, D, L], BF16, kind="ExternalInput")
    # wq/wk are dc-major: [128, dc, kc*128] so one DMA chunk unlocks a
    # whole projection output group at cold start
    wq = nc.dram_tensor("wq", [128, KC, D], BF16, kind="ExternalInput")
    wk = nc.dram_tensor("wk", [128, KC, D], BF16, kind="ExternalInput")
    # wv/wo are kc-major (moving operands)
    wv = nc.dram_tensor("wv", [128, KC, D], BF16, kind="ExternalInput")
    wo = nc.dram_tensor("wo", [128, KC, D], BF16, kind="ExternalInput")
    ones = nc.dram_tensor("ones", [128, 64], BF16, kind="ExternalInput")
    y = nc.dram_tensor("y", [NBLK, L, D], BF16, kind="ExternalOutput")
    bq = bk = bv = None
    if has_bq:
        bq = nc.dram_tensor("bq", [128, KC], F32, kind="ExternalInput")
    if has_bk:
        bk = nc.dram_tensor("bk", [128, KC], F32, kind="ExternalInput")
    if has_bv:
        bv = nc.dram_tensor("bv", [128, KC], F32, kind="ExternalInput")

    with tile.TileContext(nc) as tc:
        with (
            tc.tile_pool(name="persist", bufs=1) as pp,
            tc.tile_pool(name="ysb", bufs=3) as ypool,
            tc.tile_pool(name="rsb", bufs=2) as rpool,
            tc.tile_pool(name="scps", bufs=3, space="PSUM") as scps,
            tc.tile_pool(name="mmps", bufs=2, space="PSUM") as mmps,
        ):
            # ---- persistent SBUF tiles -------------------------------
            wq_sb = pp.tile([128, KC, D], BF16, tag="wq")
            wk_sb = pp.tile([128, KC, D], BF16, tag="wk")
            wv_sb = pp.tile([128, KC, D], BF16, tag="wv")
            wo_sb = pp.tile([128, KC, D], BF16, tag="wo")
            zt_sb = [pp.tile([128, KC, L], BF16, tag="zt%d" % i, name="zt%d" % i)
                     for i in range(2)]
            q_sb = [pp.tile([128, KC, L], BF16, tag="q%d" % i, name="q%d" % i)
                    for i in range(2)]
            k_sb = [pp.tile([128, KC, L], BF16, tag="k%d" % i, name="k%d" % i)
                    for i in range(2)]
            v_sb = [pp.tile([128, LC, VW], BF16, tag="v%d" % i, name="v%d" % i)
                    for i in range(2)]
            u_sb = [pp.tile([128, KC, L], BF16, tag="u%d" % i, name="u%d" % i)
                    for i in range(2)]
            p_e = [pp.tile([128, LC, 512], BF16, tag="pe%d" % i, name="pe%d" % i)
                   for i in range(2)]
            p_o = [pp.tile([128, LC, 512], BF16, tag="po%d" % i, name="po%d" % i)
                   for i in range(2)]

            # ---- HAM warmup --------------------------------------------
            # the framework preamble + DMA ring startup keeps the PE idle
            # for ~11us; throwaway matmuls on a zeroed tile keep it busy
            # through that window so the HAM clock gate is already at 8/8
            # (2.4 GHz) when the first projection matmul issues.  More are
            # interleaved into the DMA-paced first projection below so the
            # PE never idles long enough to re-throttle.
            wu_sb = pp.tile([128, 512], BF16, tag="wu")
            nc.vector.memset(wu_sb[:], 0.0)

            def warm(n):
                wu_ps = mmps.tile([128, 512], F32, tag="mm")
                for _ in range(n):
                    nc.tensor.matmul(
                        wu_ps[:], wu_sb[:, 0:128].opt(), wu_sb[:].opt(),
                        start=True, stop=True,
                    )

            warm(10)

            bq_sb = bk_sb = bv_sb = None
            if has_bq:
                bq_sb = pp.tile([128, KC], F32, tag="bq")
                nc.sync.dma_start(bq_sb[:], bq[:])
            if has_bk:
                bk_sb = pp.tile([128, KC], F32, tag="bk")
                nc.sync.dma_start(bk_sb[:], bk[:])
            if has_bv:
                bv_sb = pp.tile([128, KC], F32, tag="bv")
                nc.sync.dma_start(bv_sb[:], bv[:])

            # ---- initial DMAs ----------------------------------------
            # zt block 0 per-kc on the gpsimd queue (fine grain so the
            # first projection matmuls start ~1us in); wq dc-chunks on
            # the sync queue.  Remaining weights + zt follow.
            zt_r = [zt[b].rearrange("(kc p) l -> p kc l", p=128)
                    for b in range(NBLK)]
            # zt block 0 per-kc on the gpsimd queue so the first projection
            # matmuls start as soon as each chunk lands; wq/wk split across
            # the sync + scalar HWDGE rings
            for kc in range(KC):
                nc.gpsimd.dma_start(zt_sb[0][:, kc, :], zt_r[0][:, kc, :])
            for dc in range(KC):
                eng = nc.sync if dc % 2 == 0 else nc.scalar
                eng.dma_start(wq_sb[:, dc, :], wq[:, dc, :])
            for dc in range(KC):
                eng = nc.sync if dc % 2 == 0 else nc.scalar
                eng.dma_start(wk_sb[:, dc, :], wk[:, dc, :])
            for kc in range(KC):
                nc.sync.dma_start(wv_sb[:, kc, :], wv[:, kc, :])
            for kc in range(KC):
                nc.sync.dma_start(wo_sb[:, kc, :], wo[:, kc, :])
            # ones margins: head-block base (cols h*128+0:64) of every
            # (lc, h); written once per v buffer, never overwritten
            ones_b = bass.AP(
                tensor=ones[:].tensor, offset=ones[:].offset,
                ap=[list(ones[:].ap[0]), [0, H], [1, 64]],
            )
            for i in range(2):
                base = v_sb[i][:]
                for lc in range(LC):
                    dst = bass.AP(
                        tensor=base.tensor, offset=base.offset + lc * VW,
                        ap=[list(base.ap[0]), [128, H], [1, 64]],
                    )
                    nc.sync.dma_start(dst, ones_b)
            # zt block 1 prefetch (buffer 1, no prior reader)
            nc.gpsimd.dma_start(zt_sb[1][:], zt_r[1])

            # ---- emitters --------------------------------------------
            def qk_group(b, dc, which):
                """Q or K projection output-chunk dc of block b."""
                w = wq_sb if which == 0 else wk_sb
                out = q_sb[b % 2] if which == 0 else k_sb[b % 2]
                b_s = bq_sb if which == 0 else bk_sb
                ps = mmps.tile([128, L], F32, tag="mm")
                for kc in range(KC):
                    nc.tensor.matmul(
                        ps[:],
                        w[:, dc, kc * 128:(kc + 1) * 128].opt(),
                        zt_sb[b % 2][:, kc, :].opt(),
                        start=(kc == 0),
                        stop=(kc == KC - 1),
                    )
                if b_s is not None:
                    nc.scalar.activation(
                        out[:, dc, :], ps[:],
                        mybir.ActivationFunctionType.Identity,
                        bias=b_s[:, dc:dc + 1], scale=1.0,
                    )
                else:
                    nc.vector.tensor_copy(out[:, dc, :], ps[:])

            def v_group(b, g):
                """V projection group g=(lc, nh) of block b."""
                lc, nh = g // 2, g % 2
                ps = mmps.tile([128, 512], F32, tag="mm")
                for kc in range(KC):
                    nc.tensor.matmul(
                        ps[:],
                        zt_sb[b % 2][:, kc, lc * 128:(lc + 1) * 128].opt(),
                        wv_sb[:, kc, nh * 512:(nh + 1) * 512].opt(),
                        start=(kc == 0),
                        stop=(kc == KC - 1),
                    )
                # heads nh*8..nh*8+7, 64 v cols each at block offset +64
                base = v_sb[b % 2][:]
                dst = bass.AP(
                    tensor=base.tensor,
                    offset=base.offset + lc * VW + nh * 1024 + 64,
                    ap=[list(base.ap[0]), [128, 8], [1, 64]],
                )
                src = bass.AP(
                    tensor=ps.tensor, offset=ps[:].offset,
                    ap=[list(ps[:].ap[0]), [64, 8], [1, 64]],
                )
                nc.vector.tensor_copy(dst, src)

            y_rr = [0]

            def wo_group(b, g, split_y=False, y_on_scalar=False):
                """Output projection group g=(lc, eh) of block b.
                dc ascends so the accumulation chases the last u chunks.
                y leaves as bf16, round-robined over two HWDGE rings so the
                final block's writes drain ~2x faster."""
                lc, eh = g // 2, g % 2
                ps = mmps.tile([128, 512], F32, tag="mm")
                for dc in range(KC):
                    nc.tensor.matmul(
                        ps[:],
                        u_sb[b % 2][:, dc, lc * 128:(lc + 1) * 128].opt(),
                        wo_sb[:, dc, eh * 512:(eh + 1) * 512].opt(),
                        start=(dc == 0),
                        stop=(dc == KC - 1),
                    )
                y_sb = ypool.tile([128, 512], BF16, tag="y")
                halves = (0, 256, 512) if split_y else (0, 512)
                for lo, hi in zip(halves, halves[1:]):
                    if y_on_scalar:
                        nc.scalar.copy(y_sb[:, lo:hi], ps[:, lo:hi])
                    else:
                        nc.vector.tensor_copy(y_sb[:, lo:hi], ps[:, lo:hi])
                    eng = (nc.sync, nc.gpsimd)[y_rr[0] % 2]
                    y_rr[0] += 1
                    eng.dma_start(
                        y[b, lc * 128:(lc + 1) * 128,
                          eh * 512 + lo:eh * 512 + hi],
                        y_sb[:, lo:hi],
                    )

            def sc_mg(b, c, mg):
                """Scores chunk-group mg (key chunks 2mg, 2mg+1) for head
                pair c of block b; emits the even-parity exp eagerly and
                the rest after mg1 so the scalar queue drains e-major."""
                t_e = scps.tile([128, 2, 512], F32, tag="sc")
                t_o = scps.tile([128, 2, 512], F32, tag="sc")
                for i in range(2):
                    mc = 2 * mg + i
                    for par, t in ((0, t_e), (1, t_o)):
                        half = par * 64
                        nc.tensor.matmul(
                            t[:, i, :],
                            k_sb[b % 2][half:half + 64, c,
                                        mc * 128:(mc + 1) * 128].opt(),
                            q_sb[b % 2][half:half + 64, c, :].opt(),
                            start=True, stop=True,
                        )
                return t_e, t_o

            def pv(b, c, par):
                """PV matmul for head h=2c+par; returns the PSUM tile
                holding S on rows 0:64 and u on rows 64:128."""
                h = 2 * c + par
                p_t = (p_e if par == 0 else p_o)[c % 2]
                ps = mmps.tile([128, 512], F32, tag="mm")
                for mc in range(LC):
                    nc.tensor.matmul(
                        ps[:],
                        v_sb[b % 2][:, mc, h * 128:(h + 1) * 128].opt(),
                        p_t[:, mc, :].opt(),
                        start=(mc == 0), stop=(mc == LC - 1),
                    )
                return ps

            def att_phase(b, fillers):
                fi = iter(fillers)

                def F():
                    f = next(fi, None)
                    if f is not None:
                        f()

                ub = u_sb[b % 2]

                def norm(c, par, ps):
                    r = rpool.tile([64, 512], F32, tag="r%d" % par)
                    nc.vector.reciprocal_approx_fast(r[0:64, :], ps[0:64, :])
                    if par == 0:
                        nc.vector.tensor_mul(
                            ub[0:64, c, :], ps[64:128, :], r[0:64, :])
                    else:
                        nc.vector.tensor_mul(
                            ub[64:128, c, :], ps[64:128, :], r[0:64, :])
                        if has_bv:
                            nc.vector.tensor_scalar_add(
                                ub[:, c, :], ub[:, c, :], bv_sb[:, c:c + 1])

                for c in range(H // 2):
                    pe, po = (p_e[c % 2], p_o[c % 2])
                    t_e0, t_o0 = sc_mg(b, c, 0)
                    nc.scalar.activation(pe[:, 0:2, :], t_e0[:], EXP_FUNC)
                    F()
                    t_e1, t_o1 = sc_mg(b, c, 1)
                    nc.scalar.activation(pe[:, 2:4, :], t_e1[:], EXP_FUNC)
                    nc.scalar.activation(po[:, 0:2, :], t_o0[:], EXP_FUNC)
                    nc.scalar.activation(po[:, 2:4, :], t_o1[:], EXP_FUNC)
                    if c > 0:
                        norm(c - 1, 0, pv(b, c - 1, 0))
                    F()
                    if c > 0:
                        norm(c - 1, 1, pv(b, c - 1, 1))
                # epilogue: last head pair
                norm(7, 0, pv(b, 7, 0))
                F()
                norm(7, 1, pv(b, 7, 1))
                for f in fi:   # drain any leftover fillers
                    f()

            # ---- global emission order -------------------------------
            # cold: block-0 projections (DMA-paced); a few extra warmup
            # matmuls fill the early DMA-arrival gaps so the HAM never
            # sees an idle window
            for dc in range(KC):
                qk_group(0, dc, 0)
            for dc in range(KC):
                qk_group(0, dc, 1)
            for g in range(8):
                v_group(0, g)
            # zt0's last reader (V0) is emitted; buffer 0 may now be
            # refilled with block 2 (emission order IS the dep order)
            nc.gpsimd.dma_start(zt_sb[0][:], zt_r[2])

            # att0 || [Q1, K1]
            att_phase(0, [lambda dc=dc: qk_group(1, dc, 0) for dc in range(KC)]
                      + [lambda dc=dc: qk_group(1, dc, 1) for dc in range(KC)])
            for g in range(8):
                v_group(1, g)
            # zt1's last reader (V1) emitted; refill buffer 1 with block 3
            nc.gpsimd.dma_start(zt_sb[1][:], zt_r[3])
            for g in range(8):
                wo_group(0, g, y_on_scalar=True)

            # att1 || [Q2, K2]
            att_phase(1, [lambda dc=dc: qk_group(2, dc, 0) for dc in range(KC)]
                      + [lambda dc=dc: qk_group(2, dc, 1) for dc in range(KC)])
            for g in range(8):
                v_group(2, g)
            for g in range(5):
                wo_group(1, g, y_on_scalar=True)

            # att2 || [Q3, V3, K3 g0/g1] -- the trailing K3 groups land in
            # the epilogue/drain slots, just ahead of att3's first scores
            att_phase(2, [lambda dc=dc: qk_group(3, dc, 0) for dc in range(KC)]
                      + [lambda g=g: v_group(3, g) for g in range(8)]
                      + [lambda dc=dc: qk_group(3, dc, 1) for dc in range(2)])

            # att3 || [Wo1 spill, K3 rest, Wo2] -- 17 fillers so one lands
            # in the epilogue slot, pushing Wo3's first u-chunk-7 ldweights
            # past the final normalize muls
            # at most 3 Wo1 spill groups: they must all be consumed before
            # att3's first u-normalize write (sets 1 share the u buffer)
            att_phase(3, [lambda g=g: wo_group(1, g) for g in range(5, 8)]
                      + [lambda dc=dc: qk_group(3, dc, 1)
                         for dc in range(2, KC)]
                      + [lambda g=g: wo_group(2, g, y_on_scalar=(g >= 4))
                         for g in range(8)])
            for g in range(8):
                wo_group(3, g, split_y=True, y_on_scalar=True)

    nc.finalize()
    return nc


_NC_CACHE = {}


def _get_nc(flags):
    if flags not in _NC_CACHE:
        _NC_CACHE[flags] = _build_nc(*flags)
    return _NC_CACHE[flags]


def _prep(x, Wq, bq, Wk, bk, Wv, bv, Wo, bo, layer_bit):
    x = np.asarray(x, dtype=np.float32)
    C = N // CHUNK
    ids = np.arange(C)
    partner = ids ^ (1 << int(layer_bit))
    a_idx = ids[ids < partner]
    b_idx = partner[ids < partner]
    P = a_idx.shape[0]

    xr = x.reshape(B, C, CHUNK, D)
    blocks = np.concatenate([xr[:, a_idx], xr[:, b_idx]], axis=2)  # [B,P,L,D]
    blocks = np.ascontiguousarray(
        blocks.transpose(1, 0, 3, 2).reshape(P * B, D, L).astype(ml_dtypes.bfloat16)
    )  # z^T per block
    scale = np.float32(1.0 / np.sqrt(DH))

    def chunkify(vec):  # [D] -> [128, KC] chunk-major per-partition scalars
        return np.ascontiguousarray(
            np.asarray(vec, np.float32).reshape(KC, 128).T
        )

    bf = ml_dtypes.bfloat16

    def dc_major(w):  # [D, D] -> [128, dc, kc*128]
        a = np.asarray(w, np.float32).reshape(KC, 128, KC, 128)
        return np.ascontiguousarray(
            a.transpose(1, 2, 0, 3).reshape(128, KC, D).astype(bf))

    def kc_major(w):  # [D, D] -> [128, kc, D]
        a = np.asarray(w, np.float32).reshape(KC, 128, D)
        return np.ascontiguousarray(a.transpose(1, 0, 2).astype(bf))

    base = {
        "wq": dc_major(np.asarray(Wq, np.float32) * scale),
        "wk": dc_major(Wk),
        "wv": kc_major(Wv),
        "wo": kc_major(Wo),
        "ones": np.ones((128, 64), bf),
    }
    has_bq = bool(np.any(np.asarray(bq))) if bq is not None else False
    has_bk = bool(np.any(np.asarray(bk))) if bk is not None else False
    has_bv = bool(np.any(np.asarray(bv))) if bv is not None else False
    if has_bq:
        base["bq"] = chunkify(np.asarray(bq, np.float32) * scale)
    if has_bk:
        base["bk"] = chunkify(bk)
    if has_bv:
        base["bv"] = chunkify(bv)

    in_maps = []
    for core in range(NCORES):
        m = dict(base)
        m["zt"] = blocks[core * NBLK:(core + 1) * NBLK]
        in_maps.append(m)
    return in_maps, (has_bq, has_bk, has_bv), (a_idx, b_idx, P)


def _gather(results, idxs, bo):
    a_idx, b_idx, P = idxs
    yb = np.concatenate([np.asarray(r["y"], np.float32) for r in results],
                        axis=0)  # [P*B, L, D]
    yb = yb.reshape(P, B, 2, CHUNK, D)
    out = np.empty((B, N // CHUNK, CHUNK, D), np.float32)
    out[:, a_idx] = yb[:, :, 0].transpose(1, 0, 2, 3)
    out[:, b_idx] = yb[:, :, 1].transpose(1, 0, 2, 3)
    out = out.reshape(B, N, D)
    bo = np.asarray(bo, np.float32) if bo is not None else None
    if bo is not None and np.any(bo):
        out = out + bo
    return out


def _run(inputs, trace=False):
    in_maps, flags, idxs = _prep(
        inputs["x"], inputs["Wq"], inputs.get("bq"), inputs["Wk"],
        inputs.get("bk"), inputs["Wv"], inputs.get("bv"), inputs["Wo"],
        inputs.get("bo"), inputs["layer_bit"],
    )
    nc = _get_nc(flags)
    res = run_bass_kernel_spmd(nc, in_maps, list(range(NCORES)), trace=trace)
    out = _gather(res.results, idxs, inputs.get("bo"))
    return out, res


def kernel(**inputs):
    out, _ = _run(inputs, trace=False)
    return out


def kernel_traced(**inputs):
    out, res = _run(inputs, trace=True)
    return out, res

